# revision 3
# baseline (speedup 1.0000x reference)
"""Trainium2 Bass kernel for NeuralKNN (soft k-nearest-neighbors).

Reference computation (per batch element b):
    sims  = -(q . K) / sqrt(D)                      [N]
    a0    = softmax(sims)                           [N]
    repeat 16x:  w_k = softmax(a / 0.1); a += log1p(-w_k)
    out[k, f] = sum_n w_k[n] * V[f, n]              [16, F]

Strategy: pure data-parallel over B=8 -> one batch element per NeuronCore,
no collectives. Per core:
  phase 1: stream K (bf16) through the PE as stationary weights against the
           query vector -> sims laid out [128, 782] (n = t*128 + p).
  phase 2: 16 softmax iterations on [128, 782] in SBUF. Row sums come free
           via activation accum_out; cross-partition sum + broadcast via a
           ones[128,128] matmul. Stores E'_k = exp(10*a)-1 in bf16 (keeps
           precision since E ~= 1) plus a ones column.
  phase 3: V arrives block-transposed via DMA xbar transpose (bf16) as
           [n,f] tiles; one accumulating matmul per 128-n tile:
           psum[f, 0:17] += Vt.T @ [E'_0..E'_15 | 1].
  final:   out[f,k] = (psum[:,k] + psum[:,16]) * (1/S_k); host transposes.

Inputs are cast to bf16 on the host (error through the double-softmax is
~1e-5 relative; value quantization ~0.2% relative, well within tolerance)
and zero-padded from N=100000 to 100096 = 782*128.
"""

import sys

sys.path.insert(0, "/opt/trn_rl_repo")

import numpy as np
import ml_dtypes

B, D, N, F = 8, 128, 100000, 128
KK = 16
TEMP = 0.1
NT = (N + 127) // 128          # 782 n-tiles
NP = NT * 128                  # 100096 padded N
SIMS_SCALE = float(-1.0 / np.sqrt(D))
N_CORES = 8

KCH = 2048                     # keys DMA chunk (columns)
VCH = 48                       # value-transpose DMA chunk (128-col tiles)
VRING_BUFS = 10
PAD_P0 = N - (NT - 1) * 128    # first padded partition in the last tile (32)

_BF16 = ml_dtypes.bfloat16
_BUILD_CACHE = {}


def _build_nc():
    import concourse.bass as bass  # noqa: F401
    import concourse.mybir as mybir
    import concourse.tile as tile
    from concourse import bacc

    f32 = mybir.dt.float32
    bf16 = mybir.dt.bfloat16
    AF = mybir.ActivationFunctionType
    ALU = mybir.AluOpType

    nc = bacc.Bacc("TRN2", target_bir_lowering=False, debug=False)

    q_d = nc.dram_tensor("query", [D, 1], bf16, kind="ExternalInput")
    k_d = nc.dram_tensor("keys", [D, NP], bf16, kind="ExternalInput")
    v_d = nc.dram_tensor("values", [F, NP], bf16, kind="ExternalInput")
    o_d = nc.dram_tensor("out", [F, KK], f32, kind="ExternalOutput")

    with tile.TileContext(nc) as tc:
        with (
            tc.tile_pool(name="const", bufs=1) as constp,
            tc.tile_pool(name="work", bufs=1) as workp,
            tc.tile_pool(name="kring", bufs=4) as kring,
            tc.tile_pool(name="vring", bufs=VRING_BUFS) as vring,
            tc.tile_pool(name="ps_sims", bufs=2, space="PSUM") as ps_sims_p,
            tc.tile_pool(name="ps_small", bufs=2, space="PSUM") as ps_small_p,
            tc.tile_pool(name="ps_out", bufs=1, space="PSUM") as ps_out_p,
        ):
            q_sb = constp.tile([128, 1], bf16)
            nc.sync.dma_start(q_sb[:, :], q_d[:, :])
            ones = constp.tile([128, 128], f32)
            nc.vector.memset(ones[:, :], 1.0)

            sims = workp.tile([128, NT], f32)
            e_scr = workp.tile([128, NT], f32)
            term = workp.tile([128, NT], f32)
            alpha_a = workp.tile([128, NT], f32)
            alpha_b = workp.tile([128, NT], f32)
            Wp = workp.tile([128, NT, KK + 1], bf16)
            rs = workp.tile([128, 1], f32)
            recip = workp.tile([128, 1], f32)
            negr = workp.tile([128, 1], f32)
            rvec = workp.tile([128, KK], f32)
            rb_sb = workp.tile([128, KK], f32)
            out17 = workp.tile([128, KK + 1], f32)
            out_sb = workp.tile([128, KK], f32)

            # ---------------- Phase 1: sims ----------------
            ps = None
            for s in range(0, NP, KCH):
                w = min(KCH, NP - s)
                kt = kring.tile([128, KCH], bf16, tag="kt")
                nc.sync.dma_start(kt[:, 0:w], k_d[:, s : s + w])
                for j in range(w // 128):
                    t = s // 128 + j
                    c = t % 512
                    if c == 0:
                        ps = ps_sims_p.tile([128, 512], f32, tag="pss")
                    nc.tensor.matmul(
                        ps[:, c : c + 1],
                        kt[:, j * 128 : (j + 1) * 128],
                        q_sb[:, 0:1],
                        start=True,
                        stop=True,
                    )
                    if c == 511 or t == NT - 1:
                        base = (t // 512) * 512
                        nc.vector.tensor_copy(
                            sims[:, base : t + 1], ps[:, 0 : c + 1]
                        )
            # mark padded positions so exp() kills them (<=32 partitions per
            # memset when base partition is nonzero)
            for p0 in range(PAD_P0, 128, 32):
                nc.vector.memset(sims[p0 : p0 + 32, NT - 1 : NT], 1.0e5)

            # ---------------- Phase 2: iterated softmax ----------------
            # e0 = exp(-sims/sqrt(D)); rs = rowsum(e0)
            nc.scalar.activation(
                e_scr[:, :], sims[:, :], AF.Exp,
                bias=0.0, scale=SIMS_SCALE, accum_out=rs[:, 0:1],
            )
            psS = ps_small_p.tile([128, 1], f32, tag="psS")
            nc.tensor.matmul(psS[:, 0:1], ones[:, :], rs[:, 0:1], start=True, stop=True)
            nc.vector.reciprocal(recip[:, 0:1], psS[:, 0:1])
            # alpha0 = e0 / S0
            nc.scalar.mul(alpha_a[:, :], e_scr[:, :], recip[:, 0:1])
            for p0 in range(PAD_P0, 128, 32):
                nc.vector.memset(alpha_a[p0 : p0 + 32, NT - 1 : NT], -30.0)
            # the "sum of V" column
            nc.vector.memset(Wp[:, :, KK], 1.0)

            cur, nxt = alpha_a, alpha_b
            for k in range(KK):
                nc.scalar.activation(
                    e_scr[:, :], cur[:, :], AF.Exp,
                    bias=0.0, scale=1.0 / TEMP, accum_out=rs[:, 0:1],
                )
                # store E' = E - 1 (bf16)
                nc.vector.tensor_scalar_add(Wp[:, :, k], e_scr[:, :], -1.0)
                psS = ps_small_p.tile([128, 1], f32, tag="psS")
                nc.tensor.matmul(
                    psS[:, 0:1], ones[:, :], rs[:, 0:1], start=True, stop=True
                )
                nc.vector.reciprocal(recip[:, 0:1], psS[:, 0:1])
                nc.vector.tensor_copy(rvec[0:1, k : k + 1], recip[0:1, 0:1])
                if k < KK - 1:
                    nc.vector.tensor_scalar_mul(negr[:, 0:1], recip[:, 0:1], -1.0)
                    # term = ln(1 - E/S)
                    nc.scalar.activation(
                        term[:, :], e_scr[:, :], AF.Ln,
                        bias=1.0, scale=negr[:, 0:1],
                    )
                    nc.vector.tensor_add(nxt[:, :], cur[:, :], term[:, :])
                    cur, nxt = nxt, cur

            # broadcast 1/S_k across partitions: [128, KK]
            psB = ps_small_p.tile([128, KK], f32, tag="psB")
            nc.tensor.matmul(
                psB[:, :], ones[0:1, :], rvec[0:1, :], start=True, stop=True
            )
            nc.vector.tensor_copy(rb_sb[:, :], psB[:, :])

            # ---------------- Phase 3: weighted sum of values ----------------
            ps_out = ps_out_p.tile([128, KK + 1], f32)
            for s in range(0, NT, VCH):
                nt_chunk = min(VCH, NT - s)
                vt = vring.tile([128, VCH, 128], bf16, tag="vt")
                nc.sync.dma_start_transpose(
                    vt[:, 0:nt_chunk, :],
                    v_d[:, s * 128 : (s + nt_chunk) * 128],
                )
                for j in range(nt_chunk):
                    t = s + j
                    nc.tensor.matmul(
                        ps_out[:, :],
                        vt[:, j, :],
                        Wp[:, t, :],
                        start=(t == 0),
                        stop=(t == NT - 1),
                    )

            # ---------------- Final combine ----------------
            nc.vector.tensor_copy(out17[:, :], ps_out[:, :])
            nc.vector.scalar_tensor_tensor(
                out_sb[:, :],
                out17[:, 0:KK],
                out17[:, KK : KK + 1],
                rb_sb[:, :],
                op0=ALU.add,
                op1=ALU.mult,
            )
            nc.sync.dma_start(o_d[:, :], out_sb[:, :])

    nc.compile()
    return nc


def get_nc():
    if "nc" not in _BUILD_CACHE:
        _BUILD_CACHE["nc"] = _build_nc()
    return _BUILD_CACHE["nc"]


def make_in_maps(query, keys, values):
    in_maps = []
    for b in range(query.shape[0]):
        q = np.ascontiguousarray(query[b].astype(_BF16).reshape(D, 1))
        k = np.zeros((D, NP), _BF16)
        k[:, :N] = keys[b].astype(_BF16)
        v = np.zeros((F, NP), _BF16)
        v[:, :N] = values[b].astype(_BF16)
        in_maps.append({"query": q, "keys": k, "values": v})
    return in_maps


def run(query, keys, values, trace=False):
    nc = get_nc()
    from concourse.bass_utils import run_bass_kernel_spmd

    in_maps = make_in_maps(query, keys, values)
    res = run_bass_kernel_spmd(
        nc, in_maps, core_ids=list(range(N_CORES)), trace=trace
    )
    out = np.stack(
        [np.asarray(r["out"], dtype=np.float32).T for r in res.results], axis=0
    )
    return out, res


def kernel(query, keys, values):
    out, _ = run(query, keys, values, trace=False)
    return out
